# revision 1
# baseline (speedup 1.0000x reference)
"""Trainium2 Bass kernel for nn_MoE_4088808866374.

Top-1 MoE (B=4, S=1024, D=1024, E=8, F=2816, K=1) + shared expert.

The reference computes all 8 experts densely over all 4096 tokens, but the
sigmoid gate is exactly 0 for non-top-1 experts (sigmoid(-inf)), and zero
inputs propagate exactly through SwiGLU (silu(0)=0, 0*w=0). So a sparse
dispatch computes the identical result with ~4.5x fewer FLOPs.

Sharding (8 cores):
  - Expert-parallel: core e holds expert e's weights and processes the
    tokens routed to expert e (gate-scaled, capacity-padded). The
    dispatch/combine (all-to-all) is done host-side while sharding.
  - Data-parallel shared expert: core e processes tokens [512e, 512e+512)
    with the replicated shared weights.
  - Router (4096x1024x8 matmul + top-1 + sigmoid = 0.05% of total FLOPs)
    runs host-side since it determines the dispatch itself.

Device compute in float32r (PE runs it at bf16 speed for moving dim >= 256,
~2.6e-4 rel err vs 4e-3 for bf16), fp32 accumulation in PSUM.
"""

import numpy as np

import concourse.bacc as bacc
import concourse.mybir as mybir
import concourse.tile as tile
from concourse import bass_utils

# Problem constants (hardcoded per harness contract).
B, S, D, E, F = 4, 1024, 1024, 8, 2816
A = B * S            # 4096 tokens
T = A // E           # 512 shared-expert tokens per core
P = 128
D_CH = D // P        # 8
F_CH = F // P        # 22

_BUILD_CACHE = {}


def _t_chunks(n):
    """Split token count into matmul moving-dim chunks.

    float32r matmuls need moving dim >= 256 to run at full (1 cyc/row) speed;
    PSUM bank caps a chunk at 512 fp32."""
    out = []
    rem = n
    while rem > 0:
        if rem > 512:
            # keep every chunk in [256, 512]
            c = 512 if rem - 512 >= 256 or rem == 1024 else rem // 2
        else:
            c = rem
        out.append(c)
        rem -= c
    return out


def _build(cdt_name: str, C: int, reps: int = 1):
    """Build + compile the SPMD Bass kernel for capacity C routed tokens.

    reps>1 wraps the body in a hardware For_i loop (used by the test harness
    to measure per-execution device time as a slope, amortizing the ~100ms
    axon dispatch overhead)."""
    key = (cdt_name, C, reps)
    if key in _BUILD_CACHE:
        return _BUILD_CACHE[key]

    # float32r must be the declared dtype end-to-end: the BIR verifier
    # requires producers of f32r-matmul operands to round to f32r.
    sdt = getattr(mybir.dt, cdt_name)
    fp32 = mybir.dt.float32

    nc = bacc.Bacc("TRN2", target_bir_lowering=False, debug=False)

    # DRAM I/O (per core). Weight layouts are host-packed so every DMA is
    # contiguous per partition:
    #   w1p/w3p: [P(d_inner), F_CH, D_CH, P(f_inner)]
    #   w2p:     [P(f_inner), F_CH, D]
    #   x*T:     [P(d_inner), D_CH, ntok]
    xr = nc.dram_tensor("xr", [P, D_CH, C], sdt, kind="ExternalInput")
    xs = nc.dram_tensor("xs", [P, D_CH, T], sdt, kind="ExternalInput")
    w1 = nc.dram_tensor("w1", [P, F_CH, D_CH, P], sdt, kind="ExternalInput")
    w3 = nc.dram_tensor("w3", [P, F_CH, D_CH, P], sdt, kind="ExternalInput")
    w2 = nc.dram_tensor("w2", [P, F_CH, D], sdt, kind="ExternalInput")
    v1 = nc.dram_tensor("v1", [P, F_CH, D_CH, P], sdt, kind="ExternalInput")
    v3 = nc.dram_tensor("v3", [P, F_CH, D_CH, P], sdt, kind="ExternalInput")
    v2 = nc.dram_tensor("v2", [P, F_CH, D], sdt, kind="ExternalInput")
    yr = nc.dram_tensor("yr", [C, D], fp32, kind="ExternalOutput")
    ys = nc.dram_tensor("ys", [T, D], fp32, kind="ExternalOutput")
    # tiny pass-through token so the test harness can chain executions
    # back-to-back (data dependency defeats CSE / enforces ordering)
    tok = nc.dram_tensor("tok", [1, 1], fp32, kind="ExternalInput")
    tokout = nc.dram_tensor("tokout", [1, 1], fp32, kind="ExternalOutput")

    with tile.TileContext(nc) as tc:
        with tc.tile_pool(name="xpool", bufs=1) as xpool, \
             tc.tile_pool(name="wpool", bufs=5) as wpool, \
             tc.tile_pool(name="w2pool", bufs=1) as w2pool, \
             tc.tile_pool(name="midpool", bufs=1) as midpool, \
             tc.tile_pool(name="tmp", bufs=2) as tmp, \
             tc.tile_pool(name="ytmp", bufs=2) as ytmp, \
             tc.tile_pool(name="psA", bufs=2, space="PSUM") as psA, \
             tc.tile_pool(name="psB", bufs=3, space="PSUM") as psB, \
             tc.tile_pool(name="psY", bufs=3, space="PSUM") as psY:

            def swiglu(xT_d, w1_d, w3_d, w2_d, y_d, ntok, phase):
                chunks = _t_chunks(ntok)
                # activations resident; split the load per d-chunk so the
                # first matmul only waits for its own slice
                xT_sb = xpool.tile([P, D_CH, ntok], sdt, tag="x",
                                   name=f"x_{phase}")
                for d in range(D_CH):
                    nc.scalar.dma_start(xT_sb[:, d], xT_d.ap()[:, d])
                # w2 resident; slabs are prefetched inside the h-loop (they
                # are only needed by the y-phase - loading them up front
                # would queue 11.5MB of DMA ahead of the first w1 slab)
                w2_sb = w2pool.tile([P, F_CH, D], sdt, tag="w2res",
                                    name=f"w2_{phase}")
                # mid resident [P(f_inner), F_CH, ntok]
                mid_sb = midpool.tile([P, F_CH, ntok], sdt, tag="mid",
                                      name=f"mid_{phase}")

                # ---- h-phase: mid[f, t] = silu(h1) * h3 ----
                for fc in range(F_CH):
                    w1_sb = wpool.tile([P, D_CH, P], sdt, tag="w1slab",
                                       name=f"w1s_{phase}_{fc}")
                    nc.sync.dma_start(w1_sb[:], w1_d.ap()[:, fc])
                    w3_sb = wpool.tile([P, D_CH, P], sdt, tag="w3slab",
                                       name=f"w3s_{phase}_{fc}")
                    nc.sync.dma_start(w3_sb[:], w3_d.ap()[:, fc])
                    nc.sync.dma_start(w2_sb[:, fc], w2_d.ap()[:, fc])
                    t0 = 0
                    for tn in chunks:
                        ps1 = psA.tile([P, 512], fp32, tag="ps1",
                                       name=f"ps1_{phase}_{fc}_{t0}")[:, :tn]
                        for d in range(D_CH):
                            nc.tensor.matmul(
                                ps1, w1_sb[:, d],
                                xT_sb[:, d, t0:t0 + tn],
                                start=(d == 0), stop=(d == D_CH - 1))
                        ps3 = psB.tile([P, 512], fp32, tag="ps3",
                                       name=f"ps3_{phase}_{fc}_{t0}")[:, :tn]
                        for d in range(D_CH):
                            nc.tensor.matmul(
                                ps3, w3_sb[:, d],
                                xT_sb[:, d, t0:t0 + tn],
                                start=(d == 0), stop=(d == D_CH - 1))
                        silu_sb = tmp.tile([P, 512], fp32, tag="silu",
                                           name=f"silu_{phase}_{fc}_{t0}")[:, :tn]
                        nc.scalar.activation(silu_sb, ps1,
                                             mybir.ActivationFunctionType.Silu)
                        nc.vector.tensor_tensor(mid_sb[:, fc, t0:t0 + tn],
                                                silu_sb, ps3,
                                                mybir.AluOpType.mult)
                        t0 += tn

                # ---- y-phase: y[t, d] = sum_f mid[f, t] * w2[f, d] ----
                # d-slices of 352/320: N~320-352 measured ~5% faster
                # per column than N=512 on the PE
                D_SLICES = [(0, 352), (352, 352), (704, 320)]
                for tt in range((ntok + P - 1) // P):
                    tm = min(P, ntok - tt * P)  # partial last token tile
                    for ds_, (d0, dn) in enumerate(D_SLICES):
                        psy = psY.tile([P, 512], fp32, tag="psy",
                                       name=f"psy_{phase}_{tt}_{ds_}")[:tm, :dn]
                        for fc in range(F_CH):
                            nc.tensor.matmul(
                                psy, mid_sb[:, fc, tt * P:tt * P + tm],
                                w2_sb[:, fc, d0:d0 + dn],
                                start=(fc == 0), stop=(fc == F_CH - 1))
                        y_sb = ytmp.tile([P, 512], fp32, tag="ysb",
                                         name=f"y_{phase}_{tt}_{ds_}")[:tm, :dn]
                        nc.scalar.copy(y_sb, psy)
                        nc.scalar.dma_start(
                            y_d.ap()[tt * P:tt * P + tm, d0:d0 + dn],
                            y_sb)

            def body():
                swiglu(xr, w1, w3, w2, yr, C, "r")
                swiglu(xs, v1, v3, v2, ys, T, "s")

            if reps == 1:
                body()
            else:
                # staggered_reset avoids the ~2us all-engine barrier per
                # back-edge so the measured slope tracks single-shot time
                with tc.For_i(0, reps, 1, staggered_reset=True):
                    body()
            nc.sync.dma_start(tokout.ap(), tok.ap())

    nc.compile()
    _BUILD_CACHE[key] = nc
    return nc


def _sigmoid32(x):
    x = x.astype(np.float32)
    return np.where(x >= 0, 1.0 / (1.0 + np.exp(-x)),
                    np.exp(x) / (1.0 + np.exp(x))).astype(np.float32)


def _pack_w_df(w, np_dt):
    # [D, F] -> [P(d_inner), F_CH, D_CH, P(f_inner)]
    return np.ascontiguousarray(
        w.reshape(D_CH, P, F_CH, P).transpose(1, 2, 0, 3).astype(np_dt))


def _pack_w_fd(w, np_dt):
    # [F, D] -> [P(f_inner), F_CH, D]
    return np.ascontiguousarray(
        w.reshape(F_CH, P, D).transpose(1, 0, 2).astype(np_dt))


def _pack_xT(x, np_dt):
    # [n, D] -> [P(d_inner), D_CH, n]
    return np.ascontiguousarray(
        x.reshape(-1, D_CH, P).transpose(2, 1, 0).astype(np_dt))


def prepare(x_bsD, router_DE, w1_eDF, w3_eDF, w2_eFD, ws1_DF, ws3_DF, ws2_FD,
            cdt_name="float32r", C=584):
    """Host-side routing + dispatch. Returns (in_maps, aux) for the SPMD run."""
    if cdt_name == "bfloat16":
        import ml_dtypes
        np_dt = ml_dtypes.bfloat16
    else:
        np_dt = np.float32

    x = np.ascontiguousarray(np.asarray(x_bsD, np.float32).reshape(A, D))
    scores = x @ np.asarray(router_DE, np.float32)          # [A, E]
    top1 = np.argmax(scores, axis=1)                        # [A]
    gate = _sigmoid32(scores[np.arange(A), top1])           # [A]

    idx_e = [np.nonzero(top1 == e)[0] for e in range(E)]
    counts = np.array([len(i) for i in idx_e])
    while counts.max() > C:
        C += 64

    v1p = _pack_w_df(np.asarray(ws1_DF, np.float32), np_dt)
    v3p = _pack_w_df(np.asarray(ws3_DF, np.float32), np_dt)
    v2p = _pack_w_fd(np.asarray(ws2_FD, np.float32), np_dt)

    in_maps = []
    for e in range(E):
        xr = np.zeros((C, D), np.float32)
        xr[:counts[e]] = gate[idx_e[e], None] * x[idx_e[e]]
        in_maps.append({
            "xr": _pack_xT(xr, np_dt),
            "xs": _pack_xT(x[e * T:(e + 1) * T], np_dt),
            "w1": _pack_w_df(np.asarray(w1_eDF[e], np.float32), np_dt),
            "w3": _pack_w_df(np.asarray(w3_eDF[e], np.float32), np_dt),
            "w2": _pack_w_fd(np.asarray(w2_eFD[e], np.float32), np_dt),
            "v1": v1p, "v3": v3p, "v2": v2p,
            "tok": np.zeros((1, 1), np.float32),
        })
    return in_maps, (idx_e, counts, C)


def combine(results, aux):
    """Merge per-core outputs into the full [B, S, D] output."""
    idx_e, counts, C = aux
    out = np.empty((A, D), np.float32)
    for e in range(E):
        out[e * T:(e + 1) * T] = results[e]["ys"]
    for e in range(E):
        out[idx_e[e]] += results[e]["yr"][:counts[e]]
    return out.reshape(B, S, D)


def kernel(x_bsD, router_DE, w1_eDF, w3_eDF, w2_eFD, ws1_DF, ws3_DF, ws2_FD,
           cdt_name="float32r", C=584):
    in_maps, aux = prepare(x_bsD, router_DE, w1_eDF, w3_eDF, w2_eFD,
                           ws1_DF, ws3_DF, ws2_FD, cdt_name=cdt_name, C=C)
    nc = _build(cdt_name, aux[2])
    res = bass_utils.run_bass_kernel_spmd(nc, in_maps, core_ids=list(range(E)))
    return combine(res.results, aux)



# revision 3
# speedup vs baseline: 1.0736x; 1.0736x over previous
"""Trainium2 Bass kernel for nn_MoE_4088808866374.

Top-1 MoE (B=4, S=1024, D=1024, E=8, F=2816, K=1) + shared expert.

The reference computes all 8 experts densely over all 4096 tokens, but the
sigmoid gate is exactly 0 for non-top-1 experts (sigmoid(-inf)), and zero
inputs propagate exactly through SwiGLU (silu(0)=0, 0*w=0). So a sparse
dispatch computes the identical result with ~4.5x fewer FLOPs.

Sharding (8 cores, all phases SPMD — one NEFF, per-core data differs):
  - Phase R (routed, 512 slots): core e holds expert e's weights and the
    first 512 tokens routed to expert e (gate-scaled, zero-padded if
    fewer). Host-side dispatch/combine plays the role of the all-to-all.
  - Phase S (shared, 512 slots): data-parallel shared expert; core e
    processes tokens [512e, 512e+512) with replicated shared weights.
  - Phase V (overflow, tensor-parallel over F): tokens beyond slot 512 of
    hot experts. Every core processes the SAME overflow tokens but a
    DISJOINT 3-of-22 slice of the F dimension of that expert's weights
    (host packs each core's slice; zero-padded on the last core). Partial
    [D, V] outputs are summed on the host. This balances any routing skew
    without per-core program differences.
  - Router (4096x1024x8 matmul + top-1 + sigmoid = 0.05% of total FLOPs)
    runs host-side since it determines the dispatch itself.

All matmuls in bf16 (1 cyc/row at any moving size, fp32 PSUM accumulate,
~4e-3 rel err vs the 2e-2 gate), which also halves HBM traffic vs f32r.
Both GEMM phases keep the moving dim on tokens; the y-phase streams w2
d-slabs so nothing large stays resident.
"""

import numpy as np

import concourse.bacc as bacc
import concourse.mybir as mybir
import concourse.tile as tile
from concourse import bass_utils

# Problem constants (hardcoded per harness contract).
B, S, D, E, F = 4, 1024, 1024, 8, 2816
A = B * S            # 4096 tokens
R = 512              # routed slots per core (phase R)
T = 512              # shared tokens per core (phase S)
P = 128
D_CH = D // P        # 8
F_CH = F // P        # 22
FS = 3               # f-chunks per core in phase V (ceil(22/8))

_BUILD_CACHE = {}


def _build(groups: tuple, reps: int = 1):
    """Build + compile the SPMD Bass kernel.

    groups: static sizes of the per-expert overflow token groups handled by
    the F-tensor-parallel phase V (empty tuple = no overflow phase).
    reps>1 wraps the body in a hardware For_i loop (used by the test harness
    to measure per-execution device time as a slope, amortizing the ~100ms
    axon dispatch overhead)."""
    key = (groups, reps)
    if key in _BUILD_CACHE:
        return _BUILD_CACHE[key]

    bdt = mybir.dt.bfloat16
    fp32 = mybir.dt.float32
    G = len(groups)
    V = int(sum(groups))

    nc = bacc.Bacc("TRN2", target_bir_lowering=False, debug=False)

    # DRAM I/O (per core). Weight layouts are host-packed so every DMA is
    # contiguous per partition:
    #   w1/w3: [P(d_inner), F_CH, D_CH, P(f_inner)]   h-slab = [:, fc]
    #   w2:    [P(f_inner), D_CH, F_CH, P(d_inner)]   y-slab = [:, dt]
    #   x:     [P(d_inner), D_CH, ntok]
    xr = nc.dram_tensor("xr", [P, D_CH, R], bdt, kind="ExternalInput")
    xs = nc.dram_tensor("xs", [P, D_CH, T], bdt, kind="ExternalInput")
    w1r = nc.dram_tensor("w1r", [P, F_CH, D_CH, P], bdt, kind="ExternalInput")
    w3r = nc.dram_tensor("w3r", [P, F_CH, D_CH, P], bdt, kind="ExternalInput")
    w2r = nc.dram_tensor("w2r", [P, D_CH, F_CH, P], bdt, kind="ExternalInput")
    w1s = nc.dram_tensor("w1s", [P, F_CH, D_CH, P], bdt, kind="ExternalInput")
    w3s = nc.dram_tensor("w3s", [P, F_CH, D_CH, P], bdt, kind="ExternalInput")
    w2s = nc.dram_tensor("w2s", [P, D_CH, F_CH, P], bdt, kind="ExternalInput")
    yr = nc.dram_tensor("yr", [D, R], bdt, kind="ExternalOutput")
    ys = nc.dram_tensor("ys", [D, T], bdt, kind="ExternalOutput")
    if V:
        xv = nc.dram_tensor("xv", [P, D_CH, V], bdt, kind="ExternalInput")
        w1v = nc.dram_tensor("w1v", [P, G, FS, D_CH, P], bdt,
                             kind="ExternalInput")
        w3v = nc.dram_tensor("w3v", [P, G, FS, D_CH, P], bdt,
                             kind="ExternalInput")
        w2v = nc.dram_tensor("w2v", [P, G, D_CH, FS, P], bdt,
                             kind="ExternalInput")
        yv = nc.dram_tensor("yv", [D, V], fp32, kind="ExternalOutput")
    # tiny pass-through token so the test harness can chain executions
    # back-to-back (data dependency defeats CSE / enforces ordering)
    tok = nc.dram_tensor("tok", [1, 1], fp32, kind="ExternalInput")
    tokout = nc.dram_tensor("tokout", [1, 1], fp32, kind="ExternalOutput")

    with tile.TileContext(nc) as tc:
        with tc.tile_pool(name="xpool", bufs=2) as xpool, \
             tc.tile_pool(name="wpool", bufs=6) as wpool, \
             tc.tile_pool(name="w2pool", bufs=3) as w2pool, \
             tc.tile_pool(name="midpool", bufs=2) as midpool, \
             tc.tile_pool(name="vpool", bufs=1) as vpool, \
             tc.tile_pool(name="tmp", bufs=2) as tmp, \
             tc.tile_pool(name="ytmp", bufs=3) as ytmp, \
             tc.tile_pool(name="psA", bufs=2, space="PSUM") as psA, \
             tc.tile_pool(name="psB", bufs=2, space="PSUM") as psB, \
             tc.tile_pool(name="psY", bufs=3, space="PSUM") as psY:

            def swiglu(xT_d, w1_d, w3_d, w2_d, y_d, ntok, phase):
                # activations resident; split the load per d-chunk so the
                # first matmul only waits for its own slice
                xT_sb = xpool.tile([P, D_CH, ntok], bdt, tag="x",
                                   name=f"x_{phase}")
                for d in range(D_CH):
                    nc.scalar.dma_start(xT_sb[:, d], xT_d.ap()[:, d])
                # mid resident [P(f_inner), F_CH, ntok] bf16
                mid_sb = midpool.tile([P, F_CH, ntok], bdt, tag="mid",
                                      name=f"mid_{phase}")

                # ---- h-phase: mid[f, t] = silu(h1) * h3 ----
                for fc in range(F_CH):
                    w1_sb = wpool.tile([P, D_CH, P], bdt, tag="w1slab",
                                       name=f"w1s_{phase}_{fc}")
                    nc.sync.dma_start(w1_sb[:], w1_d.ap()[:, fc])
                    w3_sb = wpool.tile([P, D_CH, P], bdt, tag="w3slab",
                                       name=f"w3s_{phase}_{fc}")
                    nc.sync.dma_start(w3_sb[:], w3_d.ap()[:, fc])
                    ps1 = psA.tile([P, ntok], fp32, tag="ps1",
                                   name=f"ps1_{phase}_{fc}")
                    for d in range(D_CH):
                        nc.tensor.matmul(
                            ps1, w1_sb[:, d], xT_sb[:, d],
                            start=(d == 0), stop=(d == D_CH - 1))
                    ps3 = psB.tile([P, ntok], fp32, tag="ps3",
                                   name=f"ps3_{phase}_{fc}")
                    for d in range(D_CH):
                        nc.tensor.matmul(
                            ps3, w3_sb[:, d], xT_sb[:, d],
                            start=(d == 0), stop=(d == D_CH - 1))
                    silu_sb = tmp.tile([P, ntok], fp32, tag="silu",
                                       name=f"silu_{phase}_{fc}")
                    nc.scalar.activation(silu_sb, ps1,
                                         mybir.ActivationFunctionType.Silu)
                    nc.vector.tensor_tensor(mid_sb[:, fc], silu_sb, ps3,
                                            mybir.AluOpType.mult)

                # ---- y-phase: y[d, t] = sum_f w2[f, d] * mid[f, t] ----
                # moving dim = tokens, so no 128-token tile rounding and w2
                # streams as per-d-tile slabs (nothing large resident)
                for dt in range(D_CH):
                    w2_sb = w2pool.tile([P, F_CH, P], bdt, tag="w2slab",
                                        name=f"w2s_{phase}_{dt}")
                    nc.sync.dma_start(w2_sb[:], w2_d.ap()[:, dt])
                    psy = psY.tile([P, ntok], fp32, tag="psy",
                                   name=f"psy_{phase}_{dt}")
                    for fc in range(F_CH):
                        nc.tensor.matmul(
                            psy, w2_sb[:, fc], mid_sb[:, fc],
                            start=(fc == 0), stop=(fc == F_CH - 1))
                    y_sb = ytmp.tile([P, ntok], bdt, tag="ysb",
                                     name=f"y_{phase}_{dt}")
                    nc.scalar.copy(y_sb, psy)
                    nc.scalar.dma_start(
                        y_d.ap()[dt * P:(dt + 1) * P], y_sb)

            def overflow_phase():
                # Phase V: every core runs the same token groups against its
                # own FS-chunk slice of the hot experts' weights; host sums
                # the 8 partial outputs.
                xT_sb = xpool.tile([P, D_CH, V], bdt, tag="x", name="x_v")
                for d in range(D_CH):
                    nc.scalar.dma_start(xT_sb[:, d], xv.ap()[:, d])
                mid_sb = vpool.tile([P, G, FS, V], bdt, tag="midv",
                                    name="mid_v")
                o = 0
                for g, tg in enumerate(groups):
                    for vfc in range(FS):
                        w1_sb = wpool.tile([P, D_CH, P], bdt, tag="w1slab",
                                           name=f"w1v_{g}_{vfc}")
                        nc.sync.dma_start(w1_sb[:], w1v.ap()[:, g, vfc])
                        w3_sb = wpool.tile([P, D_CH, P], bdt, tag="w3slab",
                                           name=f"w3v_{g}_{vfc}")
                        nc.sync.dma_start(w3_sb[:], w3v.ap()[:, g, vfc])
                        ps1 = psA.tile([P, 512], fp32, tag="ps1",
                                       name=f"ps1_v_{g}_{vfc}")[:, :tg]
                        for d in range(D_CH):
                            nc.tensor.matmul(
                                ps1, w1_sb[:, d], xT_sb[:, d, o:o + tg],
                                start=(d == 0), stop=(d == D_CH - 1))
                        ps3 = psB.tile([P, 512], fp32, tag="ps3",
                                       name=f"ps3_v_{g}_{vfc}")[:, :tg]
                        for d in range(D_CH):
                            nc.tensor.matmul(
                                ps3, w3_sb[:, d], xT_sb[:, d, o:o + tg],
                                start=(d == 0), stop=(d == D_CH - 1))
                        silu_sb = tmp.tile([P, 512], fp32, tag="silu",
                                           name=f"silu_v_{g}_{vfc}")[:, :tg]
                        nc.scalar.activation(
                            silu_sb, ps1, mybir.ActivationFunctionType.Silu)
                        nc.vector.tensor_tensor(mid_sb[:, g, vfc, o:o + tg],
                                                silu_sb, ps3,
                                                mybir.AluOpType.mult)
                    o += tg

                for dt in range(D_CH):
                    psy = psY.tile([P, V], fp32, tag="psy", name=f"psy_v_{dt}")
                    o = 0
                    for g, tg in enumerate(groups):
                        w2_sb = w2pool.tile([P, FS, P], bdt, tag="w2vslab",
                                            name=f"w2v_{g}_{dt}")
                        nc.sync.dma_start(w2_sb[:], w2v.ap()[:, g, dt])
                        for vfc in range(FS):
                            nc.tensor.matmul(
                                psy[:, o:o + tg], w2_sb[:, vfc],
                                mid_sb[:, g, vfc, o:o + tg],
                                start=(vfc == 0), stop=(vfc == FS - 1))
                        o += tg
                    y_sb = ytmp.tile([P, V], fp32, tag="yvsb",
                                     name=f"yv_{dt}")
                    nc.scalar.copy(y_sb, psy)
                    nc.scalar.dma_start(yv.ap()[dt * P:(dt + 1) * P], y_sb)

            def body():
                swiglu(xr, w1r, w3r, w2r, yr, R, "r")
                swiglu(xs, w1s, w3s, w2s, ys, T, "s")
                if V:
                    overflow_phase()

            if reps == 1:
                body()
            else:
                # staggered_reset avoids the ~2us all-engine barrier per
                # back-edge so the measured slope tracks single-shot time
                with tc.For_i(0, reps, 1, staggered_reset=True):
                    body()
            nc.sync.dma_start(tokout.ap(), tok.ap())

    nc.compile()
    _BUILD_CACHE[key] = nc
    return nc


def _sigmoid32(x):
    x = x.astype(np.float32)
    return np.where(x >= 0, 1.0 / (1.0 + np.exp(-x)),
                    np.exp(x) / (1.0 + np.exp(x))).astype(np.float32)


def _np_bf16():
    import ml_dtypes
    return ml_dtypes.bfloat16


def _pack_w_df(w, np_dt):
    # [D, F] -> [P(d_inner), F_CH, D_CH, P(f_inner)]
    return np.ascontiguousarray(
        w.reshape(D_CH, P, F_CH, P).transpose(1, 2, 0, 3).astype(np_dt))


def _pack_w_fd(w, np_dt):
    # [F, D] -> [P(f_inner), D_CH, F_CH, P(d_inner)]
    return np.ascontiguousarray(
        w.reshape(F_CH, P, D_CH, P).transpose(1, 2, 0, 3).astype(np_dt))


def _pack_xT(x, np_dt):
    # [n, D] -> [P(d_inner), D_CH, n]
    return np.ascontiguousarray(
        x.reshape(-1, D_CH, P).transpose(2, 1, 0).astype(np_dt))


def prepare(x_bsD, router_DE, w1_eDF, w3_eDF, w2_eFD, ws1_DF, ws3_DF, ws2_FD):
    """Host-side routing + dispatch. Returns (in_maps, aux) for the SPMD run."""
    np_dt = _np_bf16()

    x = np.ascontiguousarray(np.asarray(x_bsD, np.float32).reshape(A, D))
    scores = x @ np.asarray(router_DE, np.float32)          # [A, E]
    top1 = np.argmax(scores, axis=1)                        # [A]
    gate = _sigmoid32(scores[np.arange(A), top1])           # [A]

    idx_e = [np.nonzero(top1 == e)[0] for e in range(E)]
    counts = np.array([len(i) for i in idx_e])

    # overflow groups: tokens beyond slot R of each hot expert, handled
    # F-tensor-parallel in phase V
    ov_experts = [e for e in range(E) if counts[e] > R]
    groups = tuple(int(counts[e] - R) for e in ov_experts)
    V = int(sum(groups))

    xg = gate[:, None] * x                                   # gate-scaled
    w1sp = _pack_w_df(np.asarray(ws1_DF, np.float32), np_dt)
    w3sp = _pack_w_df(np.asarray(ws3_DF, np.float32), np_dt)
    w2sp = _pack_w_fd(np.asarray(ws2_FD, np.float32), np_dt)

    # phase-V inputs: identical token buffer on every core; per-core weight
    # slices of FS f-chunks (zero-padded past chunk F_CH-1)
    if V:
        vx = np.concatenate([xg[idx_e[e][R:]] for e in ov_experts], axis=0)
        xvp = _pack_xT(vx, np_dt)
        w1_full = [_pack_w_df(np.asarray(w1_eDF[e], np.float32), np_dt)
                   for e in ov_experts]
        w3_full = [_pack_w_df(np.asarray(w3_eDF[e], np.float32), np_dt)
                   for e in ov_experts]
        w2_full = [_pack_w_fd(np.asarray(w2_eFD[e], np.float32), np_dt)
                   for e in ov_experts]

    in_maps = []
    for c in range(E):
        xr_ = np.zeros((R, D), np.float32)
        n = min(int(counts[c]), R)
        xr_[:n] = xg[idx_e[c][:n]]
        m = {
            "xr": _pack_xT(xr_, np_dt),
            "xs": _pack_xT(x[c * T:(c + 1) * T], np_dt),
            "w1r": _pack_w_df(np.asarray(w1_eDF[c], np.float32), np_dt),
            "w3r": _pack_w_df(np.asarray(w3_eDF[c], np.float32), np_dt),
            "w2r": _pack_w_fd(np.asarray(w2_eFD[c], np.float32), np_dt),
            "w1s": w1sp, "w3s": w3sp, "w2s": w2sp,
            "tok": np.zeros((1, 1), np.float32),
        }
        if V:
            f0 = c * FS
            w1v = np.zeros((P, len(groups), FS, D_CH, P), np_dt)
            w3v = np.zeros_like(w1v)
            w2v = np.zeros((P, len(groups), D_CH, FS, P), np_dt)
            nf = max(0, min(FS, F_CH - f0))
            for g in range(len(groups)):
                if nf > 0:
                    # w1 packed [P, F_CH, D_CH, P]: take f-chunks f0:f0+nf
                    w1v[:, g, :nf] = w1_full[g][:, f0:f0 + nf]
                    w3v[:, g, :nf] = w3_full[g][:, f0:f0 + nf]
                    # w2 packed [P(f_in), D_CH, F_CH, P(d)]: select f-chunks
                    w2v[:, g, :, :nf] = w2_full[g][:, :, f0:f0 + nf]
            m["xv"] = xvp
            m["w1v"] = w1v
            m["w3v"] = w3v
            m["w2v"] = w2v
        in_maps.append(m)
    return in_maps, (idx_e, counts, groups, ov_experts)


def combine(results, aux):
    """Merge per-core outputs into the full [B, S, D] output."""
    idx_e, counts, groups, ov_experts = aux
    out = np.empty((A, D), np.float32)
    for c in range(E):
        out[c * T:(c + 1) * T] = np.asarray(
            results[c]["ys"], np.float32).T
    for c in range(E):
        n = min(int(counts[c]), R)
        out[idx_e[c][:n]] += np.asarray(
            results[c]["yr"], np.float32).T[:n]
    if groups:
        yv = np.zeros((D, int(sum(groups))), np.float32)
        for c in range(E):
            yv += np.asarray(results[c]["yv"], np.float32)
        o = 0
        for g, e in enumerate(ov_experts):
            tg = groups[g]
            out[idx_e[e][R:R + tg]] += yv[:, o:o + tg].T
            o += tg
    return out.reshape(B, S, D)


def kernel(x_bsD, router_DE, w1_eDF, w3_eDF, w2_eFD, ws1_DF, ws3_DF, ws2_FD):
    in_maps, aux = prepare(x_bsD, router_DE, w1_eDF, w3_eDF, w2_eFD,
                           ws1_DF, ws3_DF, ws2_FD)
    nc = _build(aux[2])
    res = bass_utils.run_bass_kernel_spmd(nc, in_maps, core_ids=list(range(E)))
    return combine(res.results, aux)


# revision 27
# speedup vs baseline: 1.2464x; 1.1609x over previous
"""Trainium2 Bass kernel for nn_MoE_4088808866374.

Top-1 MoE (B=4, S=1024, D=1024, E=8, F=2816, K=1) + shared expert.

The reference computes all 8 experts densely over all 4096 tokens, but the
sigmoid gate is exactly 0 for non-top-1 experts (sigmoid(-inf)), and zero
inputs propagate exactly through SwiGLU (silu(0)=0, 0*w=0). So a sparse
dispatch computes the identical result with ~4.5x fewer FLOPs.

Sharding (8 cores, all phases SPMD — one NEFF, per-core data differs):
  - Phase R (routed, 512 slots): core e holds expert e's weights and the
    first 512 tokens routed to expert e (gate-scaled, zero-padded if
    fewer). Host-side dispatch/combine plays the role of the all-to-all.
  - Phase S (shared, 512 slots): data-parallel shared expert; core e
    processes tokens [512e, 512e+512) with replicated shared weights.
  - Phase V (overflow, tensor-parallel over F): tokens beyond slot 512 of
    hot experts. Every core processes the SAME overflow tokens but a
    DISJOINT 3-of-22 slice of the F dimension of that expert's weights
    (host packs each core's slice; zero-padded on the last core). Partial
    [D, V] outputs are summed on the host. This balances any routing skew
    without per-core program differences.
  - Router (4096x1024x8 matmul + top-1 + sigmoid = 0.05% of total FLOPs)
    runs host-side since it determines the dispatch itself.

All matmuls in bf16 (1 cyc/row at any moving size, fp32 PSUM accumulate,
~4e-3 rel err vs the 2e-2 gate), which also halves HBM traffic vs f32r.
Both GEMM phases keep the moving dim on tokens; the y-phase streams w2
d-slabs so nothing large stays resident.
"""

import numpy as np

import concourse.bacc as bacc
import concourse.mybir as mybir
import concourse.tile as tile
from concourse import bass_utils

# Problem constants (hardcoded per harness contract).
B, S, D, E, F = 4, 1024, 1024, 8, 2816
A = B * S            # 4096 tokens
R = 512              # routed slots per core (phase R)
T = 512              # shared tokens per core (phase S)
P = 128
D_CH = D // P        # 8
F_CH = F // P        # 22
FS = 3               # f-chunks per core in phase V (ceil(22/8))

_BUILD_CACHE = {}

# ablation knobs (included in the build cache key)
H_CHUNK = 512     # h-phase token moving chunk
Y_CHUNK = 512     # y-phase token moving chunk
W2_RESIDENT = 0   # 1 = load whole w2 at phase start instead of per-dt slabs
PSB_BUFS = 2
DIAG_SKIP_W3 = 0  # timing diag: reuse w1 slab for w3 (halves w1/w3 DMA)
DIAG_HALF_D = 0   # timing diag: only 4 of 8 d-chunks in h-phase matmuls


def _build(groups: tuple, reps: int = 1):
    """Build + compile the SPMD Bass kernel.

    groups: static sizes of the per-expert overflow token groups handled by
    the F-tensor-parallel phase V (empty tuple = no overflow phase).
    reps>1 wraps the body in a hardware For_i loop (used by the test harness
    to measure per-execution device time as a slope, amortizing the ~100ms
    axon dispatch overhead)."""
    key = (groups, reps, H_CHUNK, Y_CHUNK, W2_RESIDENT, PSB_BUFS,
           DIAG_SKIP_W3, DIAG_HALF_D)
    if key in _BUILD_CACHE:
        return _BUILD_CACHE[key]

    bdt = mybir.dt.bfloat16
    fp32 = mybir.dt.float32
    G = len(groups)
    V = int(sum(groups))

    nc = bacc.Bacc("TRN2", target_bir_lowering=False, debug=False)

    # DRAM I/O (per core). Weight layouts are host-packed so every DMA is
    # contiguous per partition:
    #   w1/w3: [P(d_inner), F_CH, D_CH, P(f_inner)]   h-slab = [:, fc]
    #   w2:    [P(f_inner), D_CH, F_CH, P(d_inner)]   y-slab = [:, dt]
    #   x:     [P(d_inner), D_CH, ntok]
    xr = nc.dram_tensor("xr", [P, D_CH, R], bdt, kind="ExternalInput")
    xs = nc.dram_tensor("xs", [P, D_CH, T], bdt, kind="ExternalInput")
    w1r = nc.dram_tensor("w1r", [P, F_CH, D_CH, P], bdt, kind="ExternalInput")
    w3r = nc.dram_tensor("w3r", [P, F_CH, D_CH, P], bdt, kind="ExternalInput")
    w2r = nc.dram_tensor("w2r", [P, F_CH, D], bdt, kind="ExternalInput")
    w1s = nc.dram_tensor("w1s", [P, F_CH, D_CH, P], bdt, kind="ExternalInput")
    w3s = nc.dram_tensor("w3s", [P, F_CH, D_CH, P], bdt, kind="ExternalInput")
    w2s = nc.dram_tensor("w2s", [P, F_CH, D], bdt, kind="ExternalInput")
    yr = nc.dram_tensor("yr", [R, D], bdt, kind="ExternalOutput")
    ys = nc.dram_tensor("ys", [T, D], bdt, kind="ExternalOutput")
    if V:
        # xv token groups are host-padded to 128 slots each so every V
        # matmul's moving dim >= the 128-row stationary load (no PE stalls)
        VP = G * P
        xv = nc.dram_tensor("xv", [P, D_CH, VP], bdt, kind="ExternalInput")
        w1v = nc.dram_tensor("w1v", [P, G, FS, D_CH, P], bdt,
                             kind="ExternalInput")
        w3v = nc.dram_tensor("w3v", [P, G, FS, D_CH, P], bdt,
                             kind="ExternalInput")
        w2v = nc.dram_tensor("w2v", [P, G, FS, D], bdt,
                             kind="ExternalInput")
        yv = nc.dram_tensor("yv", [V, D], fp32, kind="ExternalOutput")
    # tiny pass-through token so the test harness can chain executions
    # back-to-back (data dependency defeats CSE / enforces ordering)
    tok = nc.dram_tensor("tok", [1, 1], fp32, kind="ExternalInput")
    tokout = nc.dram_tensor("tokout", [1, 1], fp32, kind="ExternalOutput")

    with tile.TileContext(nc) as tc:
        with tc.tile_pool(name="xpool", bufs=3) as xpool, \
             tc.tile_pool(name="wpool", bufs=8) as wpool, \
             tc.tile_pool(name="w2pool", bufs=1) as w2pool, \
             tc.tile_pool(name="midpool", bufs=2) as midpool, \
             tc.tile_pool(name="vpool", bufs=1) as vpool, \
             tc.tile_pool(name="vslab", bufs=6) as vslab, \
             tc.tile_pool(name="tmp", bufs=2) as tmp, \
             tc.tile_pool(name="ytmp", bufs=3) as ytmp, \
             tc.tile_pool(name="psA", bufs=2, space="PSUM") as psA, \
             tc.tile_pool(name="psB", bufs=PSB_BUFS, space="PSUM") as psB, \
             tc.tile_pool(name="psY", bufs=3, space="PSUM") as psY:

            def swiglu(xT_d, w1_d, w3_d, w2_d, y_d, ntok, phase):
                hchunks = [(o, min(H_CHUNK, ntok - o))
                           for o in range(0, ntok, H_CHUNK)]
                # activations resident; split the load per d-chunk so the
                # first matmul only waits for its own slice
                xT_sb = xpool.tile([P, D_CH, ntok], bdt, tag="x",
                                   name=f"x_{phase}")
                for d in range(D_CH):
                    nc.scalar.dma_start(xT_sb[:, d], xT_d.ap()[:, d])
                # w2 resident; slabs are prefetched inside the h-loop (they
                # are only needed by the y-phase - loading them up front
                # would queue the whole 5.6MB of DMA ahead of the w1 slabs)
                w2_sb = w2pool.tile([P, F_CH, D], bdt, tag="w2res",
                                    name=f"w2_{phase}")
                # mid resident [P(f_inner), F_CH, ntok] bf16
                mid_sb = midpool.tile([P, F_CH, ntok], bdt, tag="mid",
                                      name=f"mid_{phase}")

                # ---- h-phase: mid[f, t] = silu(h1) * h3 ----
                n_d = D_CH // 2 if DIAG_HALF_D else D_CH
                for fc in range(F_CH):
                    w1_sb = wpool.tile([P, D_CH, P], bdt, tag="w1slab",
                                       name=f"w1s_{phase}_{fc}")
                    nc.sync.dma_start(w1_sb[:], w1_d.ap()[:, fc])
                    if DIAG_SKIP_W3:
                        w3_sb = w1_sb
                    else:
                        w3_sb = wpool.tile([P, D_CH, P], bdt, tag="w3slab",
                                           name=f"w3s_{phase}_{fc}")
                        nc.sync.dma_start(w3_sb[:], w3_d.ap()[:, fc])
                    nc.sync.dma_start(w2_sb[:, fc], w2_d.ap()[:, fc])
                    for o, tn in hchunks:
                        ps1 = psA.tile([P, H_CHUNK], fp32, tag="ps1",
                                       name=f"ps1_{phase}_{fc}_{o}")[:, :tn]
                        for d in range(n_d):
                            nc.tensor.matmul(
                                ps1, w1_sb[:, d], xT_sb[:, d, o:o + tn],
                                start=(d == 0), stop=(d == n_d - 1))
                        ps3 = psB.tile([P, H_CHUNK], fp32, tag="ps3",
                                       name=f"ps3_{phase}_{fc}_{o}")[:, :tn]
                        for d in range(n_d):
                            nc.tensor.matmul(
                                ps3, w3_sb[:, d], xT_sb[:, d, o:o + tn],
                                start=(d == 0), stop=(d == n_d - 1))
                        silu_sb = tmp.tile([P, H_CHUNK], fp32, tag="silu",
                                           name=f"silu_{phase}_{fc}_{o}")[:, :tn]
                        nc.scalar.activation(
                            silu_sb, ps1, mybir.ActivationFunctionType.Silu)
                        nc.vector.tensor_tensor(mid_sb[:, fc, o:o + tn],
                                                silu_sb, ps3,
                                                mybir.AluOpType.mult)

                # ---- y-phase: y[t, d] = sum_f mid[f, t] * w2[f, d] ----
                # d-slices of 352/320: N~320-352 measured ~5% faster
                # per column than N=512 on the PE
                D_SLICES = [(0, 352), (352, 352), (704, 320)]
                for tt in range(ntok // P):
                    for ds_, (d0, dn) in enumerate(D_SLICES):
                        psy = psY.tile([P, 512], fp32, tag="psy",
                                       name=f"psy_{phase}_{tt}_{ds_}")[:, :dn]
                        for fc in range(F_CH):
                            nc.tensor.matmul(
                                psy, mid_sb[:, fc, tt * P:(tt + 1) * P],
                                w2_sb[:, fc, d0:d0 + dn],
                                start=(fc == 0), stop=(fc == F_CH - 1))
                        y_sb = ytmp.tile([P, 512], bdt, tag="ysb",
                                         name=f"y_{phase}_{tt}_{ds_}")[:, :dn]
                        nc.scalar.copy(y_sb, psy)
                        nc.scalar.dma_start(
                            y_d.ap()[tt * P:(tt + 1) * P, d0:d0 + dn], y_sb)

            def overflow_phase():
                # Phase V: every core runs the same token groups against its
                # own FS-chunk slice of the hot experts' weights; host sums
                # the 8 partial outputs. Token slots padded to 128/group so
                # h-matmul moving dim covers the 128-row stationary load;
                # y-matmuls move over d (512 wide) with real-token stationary.
                VP = G * P
                xT_sb = xpool.tile([P, D_CH, VP], bdt, tag="x", name="x_v")
                for d in range(D_CH):
                    nc.scalar.dma_start(xT_sb[:, d], xv.ap()[:, d])
                mid_sb = vpool.tile([P, G, FS, P], bdt, tag="midv",
                                    name="mid_v")
                for g in range(G):
                    for vfc in range(FS):
                        w1_sb = wpool.tile([P, D_CH, P], bdt, tag="w1slab",
                                           name=f"w1v_{g}_{vfc}")
                        nc.sync.dma_start(w1_sb[:], w1v.ap()[:, g, vfc])
                        w3_sb = wpool.tile([P, D_CH, P], bdt, tag="w3slab",
                                           name=f"w3v_{g}_{vfc}")
                        nc.sync.dma_start(w3_sb[:], w3v.ap()[:, g, vfc])
                        ps1 = psA.tile([P, P], fp32, tag="ps1",
                                       name=f"ps1_v_{g}_{vfc}")
                        for d in range(D_CH):
                            nc.tensor.matmul(
                                ps1, w1_sb[:, d],
                                xT_sb[:, d, g * P:(g + 1) * P],
                                start=(d == 0), stop=(d == D_CH - 1))
                        ps3 = psB.tile([P, P], fp32, tag="ps3",
                                       name=f"ps3_v_{g}_{vfc}")
                        for d in range(D_CH):
                            nc.tensor.matmul(
                                ps3, w3_sb[:, d],
                                xT_sb[:, d, g * P:(g + 1) * P],
                                start=(d == 0), stop=(d == D_CH - 1))
                        silu_sb = tmp.tile([P, P], fp32, tag="silu",
                                           name=f"silu_v_{g}_{vfc}")
                        nc.scalar.activation(
                            silu_sb, ps1, mybir.ActivationFunctionType.Silu)
                        nc.vector.tensor_tensor(mid_sb[:, g, vfc],
                                                silu_sb, ps3,
                                                mybir.AluOpType.mult)

                o = 0
                for g, tg in enumerate(groups):
                    w2_sbs = []
                    for vfc in range(FS):
                        w2_sb = vslab.tile([P, D], bdt, tag="w2vslab",
                                           name=f"w2v_{g}_{vfc}")
                        nc.sync.dma_start(w2_sb[:], w2v.ap()[:, g, vfc])
                        w2_sbs.append(w2_sb)
                    for dh in range(0, D, 512):
                        psy = psY.tile([P, 512], fp32, tag="psy",
                                       name=f"psy_v_{g}_{dh}")[:tg]
                        for vfc in range(FS):
                            nc.tensor.matmul(
                                psy, mid_sb[:, g, vfc, :tg],
                                w2_sbs[vfc][:, dh:dh + 512],
                                start=(vfc == 0), stop=(vfc == FS - 1))
                        y_sb = ytmp.tile([P, 512], fp32, tag="yvsb",
                                         name=f"yv_{g}_{dh}")[:tg]
                        nc.scalar.copy(y_sb, psy)
                        nc.scalar.dma_start(
                            yv.ap()[o:o + tg, dh:dh + 512], y_sb)
                    o += tg

            def body():
                swiglu(xr, w1r, w3r, w2r, yr, R, "r")
                swiglu(xs, w1s, w3s, w2s, ys, T, "s")
                if V:
                    overflow_phase()

            if reps == 1:
                body()
            else:
                # staggered_reset avoids the ~2us all-engine barrier per
                # back-edge so the measured slope tracks single-shot time
                with tc.For_i(0, reps, 1, staggered_reset=True):
                    body()
            nc.sync.dma_start(tokout.ap(), tok.ap())

    nc.compile()
    _BUILD_CACHE[key] = nc
    return nc


def _sigmoid32(x):
    x = x.astype(np.float32)
    return np.where(x >= 0, 1.0 / (1.0 + np.exp(-x)),
                    np.exp(x) / (1.0 + np.exp(x))).astype(np.float32)


def _np_bf16():
    import ml_dtypes
    return ml_dtypes.bfloat16


def _pack_w_df(w, np_dt):
    # [D, F] -> [P(d_inner), F_CH, D_CH, P(f_inner)]
    return np.ascontiguousarray(
        w.reshape(D_CH, P, F_CH, P).transpose(1, 2, 0, 3).astype(np_dt))


def _pack_w_fd_res(w, np_dt):
    # [F, D] -> [P(f_inner), F_CH, D]  (resident w2 for the main phases)
    return np.ascontiguousarray(
        w.reshape(F_CH, P, D).transpose(1, 0, 2).astype(np_dt))


def _pack_w_fd_tp(w, np_dt):
    # [F, D] -> [P(f_inner), D_CH, F_CH, P(d_inner)]  (phase-V slab layout)
    return np.ascontiguousarray(
        w.reshape(F_CH, P, D_CH, P).transpose(1, 2, 0, 3).astype(np_dt))


def _pack_xT(x, np_dt):
    # [n, D] -> [P(d_inner), D_CH, n]
    return np.ascontiguousarray(
        x.reshape(-1, D_CH, P).transpose(2, 1, 0).astype(np_dt))


def prepare(x_bsD, router_DE, w1_eDF, w3_eDF, w2_eFD, ws1_DF, ws3_DF, ws2_FD):
    """Host-side routing + dispatch. Returns (in_maps, aux) for the SPMD run."""
    np_dt = _np_bf16()

    x = np.ascontiguousarray(np.asarray(x_bsD, np.float32).reshape(A, D))
    scores = x @ np.asarray(router_DE, np.float32)          # [A, E]
    top1 = np.argmax(scores, axis=1)                        # [A]
    gate = _sigmoid32(scores[np.arange(A), top1])           # [A]

    idx_e = [np.nonzero(top1 == e)[0] for e in range(E)]
    counts = np.array([len(i) for i in idx_e])

    # overflow groups: tokens beyond slot R of each hot expert, handled
    # F-tensor-parallel in phase V
    ov_experts = [e for e in range(E) if counts[e] > R]
    groups = tuple(int(counts[e] - R) for e in ov_experts)
    V = int(sum(groups))

    xg = gate[:, None] * x                                   # gate-scaled
    w1sp = _pack_w_df(np.asarray(ws1_DF, np.float32), np_dt)
    w3sp = _pack_w_df(np.asarray(ws3_DF, np.float32), np_dt)
    w2sp = _pack_w_fd_res(np.asarray(ws2_FD, np.float32), np_dt)

    # phase-V inputs: identical token buffer on every core; per-core weight
    # slices of FS f-chunks (zero-padded past chunk F_CH-1)
    if V:
        vx = np.zeros((len(groups) * P, D), np.float32)
        for g, e in enumerate(ov_experts):
            vx[g * P:g * P + groups[g]] = xg[idx_e[e][R:]]
        xvp = _pack_xT(vx, np_dt)
        w1_full = [_pack_w_df(np.asarray(w1_eDF[e], np.float32), np_dt)
                   for e in ov_experts]
        w3_full = [_pack_w_df(np.asarray(w3_eDF[e], np.float32), np_dt)
                   for e in ov_experts]
        w2_full = [_pack_w_fd_res(np.asarray(w2_eFD[e], np.float32), np_dt)
                   for e in ov_experts]

    in_maps = []
    for c in range(E):
        xr_ = np.zeros((R, D), np.float32)
        n = min(int(counts[c]), R)
        xr_[:n] = xg[idx_e[c][:n]]
        m = {
            "xr": _pack_xT(xr_, np_dt),
            "xs": _pack_xT(x[c * T:(c + 1) * T], np_dt),
            "w1r": _pack_w_df(np.asarray(w1_eDF[c], np.float32), np_dt),
            "w3r": _pack_w_df(np.asarray(w3_eDF[c], np.float32), np_dt),
            "w2r": _pack_w_fd_res(np.asarray(w2_eFD[c], np.float32), np_dt),
            "w1s": w1sp, "w3s": w3sp, "w2s": w2sp,
            "tok": np.zeros((1, 1), np.float32),
        }
        if V:
            f0 = c * FS
            w1v = np.zeros((P, len(groups), FS, D_CH, P), np_dt)
            w3v = np.zeros_like(w1v)
            w2v = np.zeros((P, len(groups), FS, D), np_dt)
            nf = max(0, min(FS, F_CH - f0))
            for g in range(len(groups)):
                if nf > 0:
                    # w1 packed [P, F_CH, D_CH, P]: take f-chunks f0:f0+nf
                    w1v[:, g, :nf] = w1_full[g][:, f0:f0 + nf]
                    w3v[:, g, :nf] = w3_full[g][:, f0:f0 + nf]
                    # w2 packed [P(f_in), F_CH, D]: take f-chunks f0:f0+nf
                    w2v[:, g, :nf] = w2_full[g][:, f0:f0 + nf]
            m["xv"] = xvp
            m["w1v"] = w1v
            m["w3v"] = w3v
            m["w2v"] = w2v
        in_maps.append(m)
    return in_maps, (idx_e, counts, groups, ov_experts)


def combine(results, aux):
    """Merge per-core outputs into the full [B, S, D] output."""
    idx_e, counts, groups, ov_experts = aux
    out = np.empty((A, D), np.float32)
    for c in range(E):
        out[c * T:(c + 1) * T] = np.asarray(results[c]["ys"], np.float32)
    for c in range(E):
        n = min(int(counts[c]), R)
        out[idx_e[c][:n]] += np.asarray(results[c]["yr"], np.float32)[:n]
    if groups:
        yv = np.zeros((int(sum(groups)), D), np.float32)
        for c in range(E):
            yv += np.asarray(results[c]["yv"], np.float32)
        o = 0
        for g, e in enumerate(ov_experts):
            tg = groups[g]
            out[idx_e[e][R:R + tg]] += yv[o:o + tg]
            o += tg
    return out.reshape(B, S, D)


def kernel(x_bsD, router_DE, w1_eDF, w3_eDF, w2_eFD, ws1_DF, ws3_DF, ws2_FD):
    in_maps, aux = prepare(x_bsD, router_DE, w1_eDF, w3_eDF, w2_eFD,
                           ws1_DF, ws3_DF, ws2_FD)
    nc = _build(aux[2])
    res = bass_utils.run_bass_kernel_spmd(nc, in_maps, core_ids=list(range(E)))
    return combine(res.results, aux)
